# revision 28
# baseline (speedup 1.0000x reference)
"""Sliding-window causal self-attention with RoPE on 8 Trainium2 NeuronCores.

Problem: B=2, S=2048, D=1024, H=16, HD=64, WINDOW=256, fp32.
Sharding: 2 (batch) x 4 (head-groups of 4 heads). Each core computes its
head-group's QKV projections, RoPE, windowed attention, and a partial output
projection (y_g @ Wo_g.T); the host sums the 4 partials per batch.

All matmuls/activations in float16 (tolerance is 2e-2 global-normalized;
fp16 lands ~5e-4). PSUM accumulation is fp32 throughout.

Structure (single fused pipeline):
  - Phase 1 s-chunk-major (512 cols): each x tile feeds q0/q1/k0/k1 PSUM
    accumulators + the v sweep; RoPE pair-swap via two stride-2 SBUF DMAs
    (sign folded into the sin table) + 3 fp16 vector ops.
  - qf/kf live in PER-CHUNK tiles so score key-blocks pipeline into phase 1
    as soon as their query window is projected (windows spanning a chunk
    boundary issue two matmuls into the same PSUM bank).
  - Scores land in a [128,1024] PSUM pair (one bank per head of the pair):
    one exp per key block covers both heads; multiplicative 0/1 f16 masks
    on the two triangle blocks (vector/gpsimd alternating); per-kb attn
    tiles keep chains independent.
  - AV: lhsT = v_aug [128,65] (ones column -> denominator), rhs = attn
    slices accumulated per 512-query chunk in PSUM (4-deep buffering);
    den row -> SBUF -> reciprocal -> gpsimd partition-broadcast -> one
    multiply -> yT f16.
  - Phase 3 (Wo) stile groups lag one AV query-group to hide the
    normalization chain latency; out written f16, host sums fp32 partials.
"""
import sys

for _p in ("/opt/trn_rl_repo", "/root/.axon_site/_ro/trn_rl_repo"):
    if _p not in sys.path:
        sys.path.append(_p)

import numpy as np
import concourse.bacc as bacc
import concourse.mybir as mybir
from concourse.tile import TileContext
from concourse.bass_utils import run_bass_kernel_spmd

F32 = mybir.dt.float32
F16 = mybir.dt.float16
AF = mybir.ActivationFunctionType

B, S, D = 2, 2048, 1024
H, HD = 16, 64
WINDOW = 256
THETA = 10000.0
SCALING = 1.0

HG = 4                      # head-groups (cores per batch)
HPG = H // HG               # heads per group = 4
GD = HPG * HD               # group out width = 256
NKT = D // 128              # 8 contraction tiles
NKB = S // 128              # 16 key blocks
NSC = 4                     # 512-wide s-chunks
SCALE = 1.0 / float(np.sqrt(HD))

_CACHE = {}

# score units scheduled after each phase-1 chunk: kb windows fully
# projected once chunk sc is done (window = cols [kb*128, kb*128+384))
SCHED = {0: [0, 1], 1: [2, 3, 4, 5], 2: [6, 7, 8, 9], 3: [10, 11]}
# late key blocks interleave with the AV groups (AV qq needs kb <= 2*qq+3)
LATE = {0: [12, 13], 1: [14, 15], 2: [], 3: []}


def _build():
    nc = bacc.Bacc(target_bir_lowering=False, trn_type="TRN2")

    xT = nc.dram_tensor("xT", [128, NKT * S], F16, kind="ExternalInput")
    wq = nc.dram_tensor("wq", [128, NKT * GD], F16, kind="ExternalInput")
    wk = nc.dram_tensor("wk", [128, NKT * GD], F16, kind="ExternalInput")
    wv = nc.dram_tensor("wv", [128, NKT * GD], F16, kind="ExternalInput")
    wo = nc.dram_tensor("wo", [128, GD // 128 * D], F16, kind="ExternalInput")
    cos2 = nc.dram_tensor("cos2", [128, S], F16, kind="ExternalInput")
    sin2 = nc.dram_tensor("sin2", [128, S], F16, kind="ExternalInput")
    mask4 = nc.dram_tensor("mask4", [128, 512], F16, kind="ExternalInput")
    out = nc.dram_tensor("out", [S, D], F16, kind="ExternalOutput")

    with TileContext(nc) as tc:
        with tc.tile_pool(name="const", bufs=1) as cpool, \
             tc.tile_pool(name="persist", bufs=1) as ppool:
            wq_sb = cpool.tile([128, NKT, GD], F16)
            wk_sb = cpool.tile([128, NKT, GD], F16)
            wv_sb = cpool.tile([128, NKT, GD], F16)
            wo_sb = cpool.tile([128, GD // 128, D], F16)
            cos_sb = cpool.tile([128, S], F16)
            sin_sb = cpool.tile([128, S], F16)
            mask_sb = cpool.tile([128, 2, 2, 128], F16)
            nc.scalar.dma_start(wq_sb[:].rearrange("p a b -> p (a b)"), wq.ap())
            nc.scalar.dma_start(wk_sb[:].rearrange("p a b -> p (a b)"), wk.ap())
            nc.scalar.dma_start(wv_sb[:].rearrange("p a b -> p (a b)"), wv.ap())
            nc.scalar.dma_start(cos_sb[:], cos2[:])
            nc.scalar.dma_start(sin_sb[:], sin2[:])
            nc.scalar.dma_start(
                mask_sb[:].rearrange("p a b c -> p (a b c)"), mask4.ap())

            # persistent activations; denominator ones column via memset
            v_sb = ppool.tile([128, NKB * HPG * 65], F16)
            nc.gpsimd.memset(
                v_sb[:].rearrange("p (g c) -> p g c", c=65)[:, :, 64], 1.0)

            # q/k in per-chunk tiles so scores can pipeline into phase 1
            qfc = [[ppool.tile([128, 512], F16, name=f"qf{t}_{c}")
                    for c in range(NSC)] for t in range(2)]
            kfc = [[ppool.tile([128, 512], F16, name=f"kf{t}_{c}")
                    for c in range(NSC)] for t in range(2)]
            yT = [ppool.tile([128, S], F16, name=f"yT{t}") for t in range(2)]

            with tc.tile_pool(name="attn", bufs=1) as apool, \
                 tc.tile_pool(name="smalls", bufs=4) as spool, \
                 tc.tile_pool(name="p3sb", bufs=3) as opool:
                attns = [[apool.tile([128, 768], F16, name=f"attn{th}_{kb}")
                          for kb in range(NKB)] for th in range(2)]
                u_cnt = [0]

                def emit_score(th, kb, scps):
                    q0 = kb * 128
                    n = min(384, S - q0)
                    sc_t = scps.tile([128, 1024], F32, tag="sc",
                                     name=f"sc{th}_{kb}")
                    c0, off0 = q0 // 512, q0 % 512
                    w0 = min(n, 512 - off0)
                    pieces = [(c0, off0, 0, w0)]
                    if w0 < n:
                        pieces.append((c0 + 1, 0, w0, n - w0))
                    for i in range(2):
                        ph = 64 * i
                        for (cc, coff, aoff, w) in pieces:
                            nc.tensor.matmul(
                                sc_t[:, i * 512 + aoff:i * 512 + aoff + w],
                                kfc[th][c0][ph:ph + 64, off0:off0 + 128],
                                qfc[th][cc][ph:ph + 64, coff:coff + w],
                                start=True, stop=True)
                    scv = sc_t[:].rearrange("p (g c) -> p g c", g=2)
                    at = attns[th][kb]
                    atv = at[:].rearrange("p (g c) -> p g c", g=2)
                    nc.scalar.activation(atv[:, :, 0:n], scv[:, :, 0:n],
                                         AF.Exp, scale=SCALE)
                    # multiplicative 0/1 mask on the two triangle blocks
                    eng = nc.gpsimd if u_cnt[0] % 2 == 1 else nc.vector
                    u_cnt[0] += 1
                    if n == 384:
                        at4 = at[:].rearrange("p (g b c) -> p g b c",
                                              g=2, b=3)[:, :, 0::2, :]
                        eng.tensor_mul(at4, at4, mask_sb[:])
                    else:
                        eng.tensor_mul(atv[:, :, 0:128], atv[:, :, 0:128],
                                       mask_sb[:, :, 0, :])

                # -------- phase 1 + pipelined scores --------
                with tc.tile_pool(name="scps", bufs=1, space="PSUM") as scps:
                    with tc.tile_pool(name="p1x", bufs=1) as xpool, \
                         tc.tile_pool(name="p1raw", bufs=3) as rawpool, \
                         tc.tile_pool(name="p1tmp", bufs=3) as tpool, \
                         tc.tile_pool(name="p1acc", bufs=1, space="PSUM") as psa, \
                         tc.tile_pool(name="p1v", bufs=1, space="PSUM") as psv:
                        xt = [[xpool.tile([128, 512], F16, name=f"x{kt}_{sc}")
                               for sc in range(NSC)] for kt in range(NKT)]
                        for sc in range(NSC):
                            eng = nc.sync if sc < 2 else nc.gpsimd
                            for kt in range(NKT):
                                eng.dma_start(
                                    xt[kt][sc][:],
                                    xT.ap()[:, kt * S + sc * 512:
                                            kt * S + sc * 512 + 512])
                        nc.sync.dma_start(
                            wo_sb[:].rearrange("p a b -> p (a b)"), wo.ap())

                        wsel = [(wq_sb, 0, qfc[0]), (wq_sb, 128, qfc[1]),
                                (wk_sb, 0, kfc[0]), (wk_sb, 128, kfc[1])]
                        for sc in range(NSC):
                            s0 = sc * 512
                            accs = [psa.tile([128, 512], F32, tag=f"acc{t}",
                                             name=f"acc{sc}_{t}")
                                    for t in range(4)]
                            for kt in range(NKT):
                                st, sp = (kt == 0), (kt == NKT - 1)
                                for t, (w_t, off, dst) in enumerate(wsel):
                                    nc.tensor.matmul(
                                        accs[t][:], w_t[:, kt, off:off + 128],
                                        xt[kt][sc][:], start=st, stop=sp)
                            # v sweep: x chunks stationary, wv moving
                            vaccs = [psv.tile([128, 512], F32, tag=f"vacc{j}",
                                              name=f"vacc{sc}_{j}")
                                     for j in range(2)]
                            for kt in range(NKT):
                                st, sp = (kt == 0), (kt == NKT - 1)
                                for j in range(2):
                                    for jj in range(2):
                                        sb = 2 * j + jj
                                        nc.tensor.matmul(
                                            vaccs[j][:, jj * 256:(jj + 1) * 256],
                                            xt[kt][sc][:, sb * 128:(sb + 1) * 128],
                                            wv_sb[:, kt, 0:256],
                                            start=(st and jj == 0), stop=sp)
                            # rope evacuation: pair-swap across partitions via
                            # two stride-2 SBUF DMAs (sign folded into sinS)
                            for t, (w_t, off, dst) in enumerate(wsel):
                                raw = rawpool.tile([128, 512], F16, tag="raw")
                                nc.scalar.copy(raw[:], accs[t][:])
                                rot = rawpool.tile([128, 512], F16, tag="rot")
                                nc.sync.dma_start(rot[0:127:2, :],
                                                  raw[1:128:2, :])
                                nc.sync.dma_start(rot[1:128:2, :],
                                                  raw[0:127:2, :])
                                t1 = tpool.tile([128, 512], F16, tag="t1")
                                nc.vector.tensor_mul(t1[:], rot[:],
                                                     sin_sb[:, s0:s0 + 512])
                                t2 = tpool.tile([128, 512], F16, tag="t2")
                                nc.gpsimd.tensor_mul(t2[:], raw[:],
                                                     cos_sb[:, s0:s0 + 512])
                                nc.vector.tensor_add(dst[sc][:], t1[:], t2[:])
                            # v evacuation into the 65-stride layout
                            for j in range(2):
                                for jj in range(2):
                                    kb = sc * 4 + 2 * j + jj
                                    dstv = v_sb[:, kb * HPG * 65:
                                                (kb + 1) * HPG * 65]
                                    nc.vector.tensor_copy(
                                        dstv.rearrange("p (g c) -> p g c",
                                                       c=65)[:, :, 0:64],
                                        vaccs[j][:, jj * 256:(jj + 1) * 256]
                                        .rearrange("p (g c) -> p g c", c=64))
                            # pipelined score units for this chunk
                            for kb in SCHED[sc]:
                                for th in range(2):
                                    emit_score(th, kb, scps)

                # -------- AV + output projection (lag-1 stile groups) -----
                with tc.tile_pool(name="avps", bufs=4, space="PSUM") as avps, \
                     tc.tile_pool(name="p3ps", bufs=2, space="PSUM") as ps3, \
                     tc.tile_pool(name="avsc", bufs=1, space="PSUM") as avscps:
                    u = 0

                    def emit_p3(stile):
                        r0 = stile * 128
                        ot = opool.tile([128, D], F16, tag="ot")
                        for dc in range(2):
                            oacc = ps3.tile([128, 512], F32, tag="oacc")
                            for ct in range(2):
                                nc.tensor.matmul(
                                    oacc[:], yT[ct][:, r0:r0 + 128],
                                    wo_sb[:, ct, dc * 512:(dc + 1) * 512],
                                    start=(ct == 0), stop=(ct == 1))
                            if dc == 0:
                                nc.scalar.copy(ot[:, 0:512], oacc[:])
                            else:
                                nc.vector.tensor_copy(ot[:, 512:1024], oacc[:])
                            nc.sync.dma_start(
                                out.ap()[r0:r0 + 128, dc * 512:(dc + 1) * 512],
                                ot[:, dc * 512:(dc + 1) * 512])

                    for qq in range(4):
                        for kb in LATE[qq]:
                            for th in range(2):
                                emit_score(th, kb, avscps)
                        for th in range(2):
                            for i in range(2):
                                h = 2 * th + i
                                ph = 64 * i
                                acc = avps.tile([65, 512], F32, tag="av",
                                                name=f"av{th}_{i}_{qq}")
                                first = True
                                for j2 in range(2):      # 256-q halves
                                    m = 2 * qq + j2
                                    qb0 = 2 * m
                                    mms = []
                                    if m >= 1:
                                        mms.append((qb0 - 2, 0, 256, 128))
                                        mms.append((qb0 - 1, 0, 128, 256))
                                        mms.append((qb0, 0, 0, 256))
                                    else:
                                        mms.append((qb0, 0, 0, 256))
                                    mms.append((qb0 + 1, 128, 0, 128))
                                    for ii, (kb, jo, ao, w) in enumerate(mms):
                                        wdt = min(w, S - kb * 128 - ao)
                                        nc.tensor.matmul(
                                            acc[:, j2 * 256 + jo:
                                                j2 * 256 + jo + wdt],
                                            v_sb[:, (kb * HPG + h) * 65:
                                                 (kb * HPG + h) * 65 + 65],
                                            attns[th][kb][:, i * 384 + ao:
                                                          i * 384 + ao + wdt],
                                            start=first,
                                            stop=(j2 == 1
                                                  and ii == len(mms) - 1))
                                        first = False
                                den = spool.tile([1, 512], F32, tag="den")
                                if u % 2 == 0:
                                    nc.scalar.copy(den[:], acc[64:65, :])
                                else:
                                    nc.vector.tensor_copy(den[:],
                                                          acc[64:65, :])
                                u += 1
                                rc0 = spool.tile([1, 512], F32, tag="rc0")
                                nc.vector.reciprocal_approx_fast(
                                    out=rc0[:], in_=den[:])
                                rbs = spool.tile([64, 512], F32, tag="rbs")
                                nc.gpsimd.partition_broadcast(rbs[:], rc0[:])
                                nc.vector.tensor_mul(
                                    yT[th][ph:ph + 64,
                                           qq * 512:(qq + 1) * 512],
                                    acc[0:64, :], rbs[:])
                        if qq >= 1:
                            for stile in range(4 * (qq - 1), 4 * qq):
                                emit_p3(stile)
                    for stile in range(12, 16):
                        emit_p3(stile)

    nc.finalize()
    return nc


def _rope_tables():
    inv_freq = 1.0 / (THETA ** (np.arange(0, HD, 2, dtype=np.float64) / HD))
    t = np.arange(S, dtype=np.float64) / max(SCALING, 1e-6)
    freqs = np.outer(t, inv_freq)                      # [S, HD/2]
    emb = np.concatenate((freqs, freqs), axis=-1)      # [S, HD]
    return np.cos(emb), np.sin(emb)


def _swz(w):
    # [kt*128, X] -> [128, kt*X] partition-major contiguous
    kt = w.shape[0] // 128
    return np.ascontiguousarray(
        w.reshape(kt, 128, w.shape[1]).transpose(1, 0, 2).reshape(128, -1))


def _host_prep(x, Wq, Wk, Wv, Wo):
    cos, sin = _rope_tables()
    cosT2 = np.ascontiguousarray(np.tile(cos.T, (2, 1))).astype(np.float16)
    sinT2 = np.ascontiguousarray(np.tile(sin.T, (2, 1))).astype(np.float16)
    # fold the rotate-half signs into the sin table: rot[2i] = -raw[2i+1],
    # rot[2i+1] = +raw[2i]; the DMA pair-swap moves values unsigned
    sinT2[0::2, :] *= -1.0

    # mask4 [128 key, (i=2, block=2, 128 col)]: block 0 = causal triangle of
    # the kb-aligned window block, block 1 = far-window triangle (col+256)
    cc = np.arange(128)[None, :]
    kk = np.arange(128)[:, None]
    mb0 = (cc >= kk).astype(np.float16)        # [128, 128]
    mb1 = (cc < kk).astype(np.float16)
    mi = np.concatenate([mb0, mb1], axis=1)    # [128, 256]
    m4 = np.ascontiguousarray(np.concatenate([mi, mi], axis=1))  # [128, 512]

    in_maps = []
    for c in range(8):
        b, g = c // HG, c % HG
        gsl = slice(g * GD, (g + 1) * GD)
        in_maps.append({
            "xT": _swz(x[b].T.astype(np.float16).reshape(D, S)),
            "wq": _swz(Wq[gsl, :].T).astype(np.float16),
            "wk": _swz(Wk[gsl, :].T).astype(np.float16),
            "wv": _swz(Wv[gsl, :].T).astype(np.float16),
            "wo": _swz(Wo[:, gsl].T).astype(np.float16),
            "cos2": cosT2, "sin2": sinT2, "mask4": m4,
        })
    return in_maps


def _run(inputs, trace=False, **kw):
    if "nc" not in _CACHE:
        _CACHE["nc"] = _build()
    in_maps = _host_prep(inputs["x"], inputs["Wq"], inputs["Wk"],
                         inputs["Wv"], inputs["Wo"])
    return run_bass_kernel_spmd(_CACHE["nc"], in_maps, list(range(8)),
                                trace=trace, **kw)


def kernel(x, Wq, Wk, Wv, Wo):
    res = _run({"x": x, "Wq": Wq, "Wk": Wk, "Wv": Wv, "Wo": Wo})
    out = np.zeros((B, S, D), dtype=np.float32)
    for c in range(8):
        out[c // HG] += res.results[c]["out"].astype(np.float32)
    return out
